# revision 17
# baseline (speedup 1.0000x reference)
"""CSWin self-attention Trainium2 kernel (v5).

Sharding: data-parallel over batch B=8 across 8 cores (1 image per core).
Per-core pipeline (image = 128x128 spatial, C=256):
  A) x loaded HBM->SBUF bf16 via SWDGE cast-DMA in 8 chunks (16 token-tiles
     each). LN stats via bn_stats/bn_aggr; rstd via Quake-style rsqrt on DVE.
     Normalize into a contiguous per-chunk buffer, then ONE batched xbar
     DMA-transpose per chunk (1 MB) into the unified channel-major y^T store
     (layout: col = tile*256 + half*128 + tok).
  B) Per direction (H interleaved with phase-A chunks, then V), per stripe:
     qkv into a single [128,768] psum tile (q|k|v), ONE merged drain to bf16
     SBUF (alternating DVE/ScalarE to balance engine load vs the exp),
     S^T row-tiled 4 heads (K=32) into two [128,1024] psum halves, exp per
     half on ScalarE, attn@V col-tiled per head (M=32) plus ones-matmul
     denominators into one [128,512] psum, reciprocal_approx_fast + mul
     straight into hHt/hVt.
  C) Projection in groups of 8 token-tiles: matmuls into a [128,2048] psum,
     x group prefetched fp32 via SWDGE, residual add on DVE, one batched
     store per group.
"""

import numpy as np
import ml_dtypes

import concourse.bass as bass
import concourse.bacc as bacc
import concourse.mybir as mybir
import concourse.tile as tile
from concourse.bass_utils import run_bass_kernel_spmd

F32 = mybir.dt.float32
BF16 = mybir.dt.bfloat16
I32 = mybir.dt.int32
AF = mybir.ActivationFunctionType
ALU = mybir.AluOpType

B = 8
HH = 128
WW = 128
C = 256
T = HH * WW          # 16384 tokens
NT = T // 128        # 128 token tiles
NCHUNK = 8           # phase A chunks
TPC = NT // NCHUNK   # 16 tiles per chunk
NS = 64              # stripes per direction
SEQ = 256
NHD = 4
HD = 32
SCALE = HD ** -0.5
EPS = 1e-5


def build_nc(has_qbias: bool, has_pbias: bool) -> bass.Bass:
    nc = bacc.Bacc("TRN2", target_bir_lowering=False, debug=False)
    x_h = nc.dram_tensor("x", [T, C], F32, kind="ExternalInput")
    wqkv_h = nc.dram_tensor("wqkv", [2, 128, 768], BF16, kind="ExternalInput")
    wproj_h = nc.dram_tensor("wproj", [2, 128, 256], BF16, kind="ExternalInput")
    bqkv_h = nc.dram_tensor("bqkv", [1, 768], BF16, kind="ExternalInput")
    bproj_h = nc.dram_tensor("bproj", [1, 256], BF16, kind="ExternalInput")
    out_h = nc.dram_tensor("out", [T, C], F32, kind="ExternalOutput")

    with tile.TileContext(nc) as tc, tc.tile_pool(name="persist", bufs=1) as pp:
        # ---------------- persistent SBUF ----------------
        # y^T store: column j = tile*256 + half*128 + tok  (tile = token tile)
        yt = pp.tile([128, 2 * T], BF16, name="yt", tag="yt")
        hHt = pp.tile([128, T], BF16, name="hHt", tag="hHt")
        hVt = pp.tile([128, T], BF16, name="hVt", tag="hVt")
        wqkv = pp.tile([128, 2 * 768], BF16, name="wqkv", tag="wqkv")
        wproj = pp.tile([128, 2 * 256], BF16, name="wproj", tag="wproj")
        brow = pp.tile([1, 768], BF16, name="brow", tag="brow")
        bprow = pp.tile([1, 256], BF16, name="bprow", tag="bprow")
        ones = pp.tile([1, 256], BF16, name="ones", tag="ones")
        ones32 = pp.tile([128, 32], BF16, name="ones32", tag="ones32")

        nc.sync.dma_start(out=wqkv[:, 0:768], in_=wqkv_h[0])
        nc.sync.dma_start(out=wqkv[:, 768:1536], in_=wqkv_h[1])
        nc.sync.dma_start(out=wproj[:, 0:256], in_=wproj_h[0])
        nc.sync.dma_start(out=wproj[:, 256:512], in_=wproj_h[1])
        if has_qbias:
            nc.sync.dma_start(out=brow[:], in_=bqkv_h[:])
        if has_pbias:
            nc.sync.dma_start(out=bprow[:], in_=bproj_h[:])
        nc.vector.memset(ones[:], 1.0)
        # PE/HAM warmup: build ones32 via 64 accumulating outer-product
        # matmuls of exact power-of-two values (64 * 2^-6 = 1.0). Runs
        # during the phase-A LN lead-in so the PE clock is at 8/8 by the
        # first stripe; every matmul feeds the live ones32 value (no DCE).
        c64 = pp.tile([1, 128], BF16, name="c64", tag="c64")
        nc.vector.memset(c64[:], 1.0 / 64.0)

        # [128, tile, 256] view of the y^T store
        yt3 = yt[:].rearrange("p (t x) -> p t x", x=256)

        def stripe_qkv(di, g, drain_on_act, qkv_pool, ds_pool):
            horiz = di == 0
            qoff = 0 if horiz else 128
            # qkv rhs views: token order (h-outer, w-inner) for H;
            # (w-outer, h-inner) for V.
            if horiz:
                rv = [yt3[:, 2 * g:2 * g + 2, kc * 128:kc * 128 + 128]
                      for kc in range(2)]
                # v lhsT: y^T tile [128 ch, 128 tok] for row 2g+sc
                vw = [[yt3[:, 2 * g + sc, kc * 128:kc * 128 + 128]
                       for kc in range(2)] for sc in range(2)]
            else:
                rv = [yt3[:, :, kc * 128 + 2 * g:kc * 128 + 2 * g + 2]
                      .rearrange("p t w -> p w t") for kc in range(2)]
                vw = [[yt3[:, :, kc * 128 + 2 * g + sc]
                       for kc in range(2)] for sc in range(2)]
            # ---- qkv into one [128, 768] psum (q | k | v0 | v1) ----
            qkv_ps = qkv_pool.tile([128, 768], F32, tag="qkvps")
            for qk in range(2):  # 0 = q, 1 = k
                col = qk * 256
                woff = qoff + qk * 256
                for kc in range(2):
                    nc.tensor.matmul(
                        qkv_ps[:, col:col + 256],
                        lhsT=wqkv[:, kc * 768 + woff:kc * 768 + woff + 128],
                        rhs=rv[kc], start=kc == 0,
                        stop=kc == 1 and not has_qbias)
                if has_qbias:
                    nc.tensor.matmul(
                        qkv_ps[:, col:col + 256],
                        lhsT=brow[:, woff:woff + 128],
                        rhs=ones[:, 0:256], start=False, stop=True)
            for sc in range(2):
                for kc in range(2):
                    nc.tensor.matmul(
                        qkv_ps[:, 512 + sc * 128:640 + sc * 128],
                        lhsT=vw[sc][kc],
                        rhs=wqkv[:, kc * 768 + 512 + qoff:kc * 768 + 640 + qoff],
                        start=kc == 0, stop=kc == 1 and not has_qbias)
                if has_qbias:
                    nc.tensor.matmul(
                        qkv_ps[:, 512 + sc * 128:640 + sc * 128],
                        lhsT=ones[:, 0:128],
                        rhs=brow[:, 512 + qoff:640 + qoff],
                        start=False, stop=True)
            # ---- merged drain (q|k|v) -> bf16 SBUF ----
            ds = ds_pool.tile([128, 768], BF16, tag="ds")
            if drain_on_act:
                nc.scalar.activation(ds[:], qkv_ps[:], AF.Copy)
            else:
                nc.vector.tensor_copy(ds[:], qkv_ps[:])
            return ds

        def stripe_sT(di, g, ds, s_pool, esb_pool):
            # ---- S^T (row-tiled 4 heads, K=32), one [128,2048] psum ----
            # order: sc-outer, head-inner so adjacent MMs hit different
            # row groups (FIFO starts stay concurrent)
            e_sb = esb_pool.tile([128, 2048], BF16, tag="esb")
            s_ps = s_pool.tile([128, 2048], F32, tag="sps")
            for sc in range(2):
                for h in range(NHD):
                    nc.tensor.matmul(
                        s_ps[:, h * 512 + sc * 256:h * 512 + sc * 256 + 256],
                        lhsT=ds[32 * h:32 * h + 32,
                                256 + sc * 128:384 + sc * 128],
                        rhs=ds[32 * h:32 * h + 32, 0:256],
                        start=True, stop=True,
                        tile_position=(32 * h, 0))
            nc.scalar.activation(e_sb[:], s_ps[:], AF.Exp, scale=SCALE)
            return e_sb

        def stripe_back(di, g, ds, e_sb, od_pool, drec_pool):
            horiz = di == 0
            hdst = hHt if horiz else hVt
            # ---- attn @ V col-tiled per head (M=32) + denominators ----
            # phase order: all 4 o-MMs (distinct col groups, concurrent),
            # then all 4 d-MMs — avoids per-MM weight-buffer serialization
            od_ps = od_pool.tile([128, 512], F32, tag="odps")
            for sc in range(2):
                for h in range(NHD):
                    nc.tensor.matmul(
                        od_ps[32 * h:32 * h + 32, 0:256],
                        lhsT=ds[:, 512 + sc * 128 + 32 * h:
                                544 + sc * 128 + 32 * h],
                        rhs=e_sb[:, h * 512 + sc * 256:
                                 h * 512 + sc * 256 + 256],
                        start=sc == 0, stop=sc == 1,
                        tile_position=(0, 32 * h), skip_group_check=True)
                for h in range(NHD):
                    nc.tensor.matmul(
                        od_ps[32 * h:32 * h + 32, 256:512],
                        lhsT=ones32[:],
                        rhs=e_sb[:, h * 512 + sc * 256:
                                 h * 512 + sc * 256 + 256],
                        start=sc == 0, stop=sc == 1,
                        tile_position=(0, 32 * h), skip_group_check=True)
            # ---- normalize straight into h^T ----
            drec = drec_pool.tile([128, 256], F32, tag="drec")
            nc.vector.reciprocal_approx_fast(drec[:], od_ps[:, 256:512])
            nc.vector.tensor_mul(
                hdst[:, g * 256:(g + 1) * 256], od_ps[:, 0:256], drec[:])

        # ---------------- phases A+B ----------------
        with (
            tc.tile_pool(name="xch", bufs=3) as xch_pool,
            tc.tile_pool(name="ynrm", bufs=2) as ynrm_pool,
            tc.tile_pool(name="stat", bufs=2) as stat_pool,
            tc.tile_pool(name="qkvps", bufs=1, space="PSUM") as qkv_pool,
            tc.tile_pool(name="sps", bufs=1, space="PSUM") as s_pool,
            tc.tile_pool(name="odps", bufs=2, space="PSUM") as od_pool,
            tc.tile_pool(name="ds", bufs=3) as ds_pool,
            tc.tile_pool(name="esb", bufs=3) as esb_pool,
            tc.tile_pool(name="drec", bufs=2) as drec_pool,
        ):
            # PE/HAM warmup + exact ones32 (64 * 2^-6 = 1.0)
            wm_ps = od_pool.tile([128, 512], F32, tag="odps")
            for i in range(64):
                nc.tensor.matmul(
                    wm_ps[:, 0:32], lhsT=c64[:, 0:128], rhs=ones[0:1, 0:32],
                    start=i == 0, stop=i == 63)
            nc.vector.tensor_copy(ones32[:], wm_ps[:, 0:32])

            pend = None  # (di, g, ds, e_sb) awaiting attn@V

            def run_stripe(di, g, drain_on_act):
                # PE stream: qkv(g) -> attnV(g-1) -> S^T(g); keeps the PE
                # FIFO head from stalling on the qkv drain while attnV of
                # the previous stripe is already runnable.
                nonlocal pend
                ds = stripe_qkv(di, g, drain_on_act, qkv_pool, ds_pool)
                if pend is not None:
                    stripe_back(*pend, od_pool, drec_pool)
                e_sb = stripe_sT(di, g, ds, s_pool, esb_pool)
                pend = (di, g, ds, e_sb)

            def chunk_load(ch):
                xch = xch_pool.tile([128, TPC, 256], BF16, tag="xch")
                nc.gpsimd.dma_start(
                    out=xch[:],
                    in_=x_h[ch * TPC * 128:(ch + 1) * TPC * 128, :]
                    .rearrange("(t p) c -> p t c", t=TPC))
                return xch

            def stats_alloc():
                st = stat_pool.tile([128, TPC, 6], F32, tag="st")
                mv = stat_pool.tile([128, TPC, 2], F32, tag="mv")
                return st, mv

            def stats_piece(xch, st, mv, t0, n):
                for t in range(t0, t0 + n):
                    nc.vector.bn_stats(st[:, t, :], xch[:, t, :])
                    nc.vector.bn_aggr(mv[:, t, :], st[:, t, :])

            def chunk_finish(ch, xch, mv, t0, ntile):
                # rstd = (var+eps)^-1/2 (Quake seed + 2 NR), normalize, and
                # one batched xbar transpose for tiles [t0, t0+ntile)
                v1 = stat_pool.tile([128, TPC], F32, tag="v1")
                r0 = stat_pool.tile([128, TPC], F32, tag="r0")
                aa = stat_pool.tile([128, TPC], F32, tag="aa")
                uu = stat_pool.tile([128, TPC], F32, tag="uu")
                r1 = stat_pool.tile([128, TPC], F32, tag="r1")
                rstd = stat_pool.tile([128, TPC], F32, tag="rstd")
                nv = v1[:, 0:ntile]
                na = aa[:, 0:ntile]
                nu = uu[:, 0:ntile]
                nc.vector.tensor_scalar_add(nv, mv[:, t0:t0 + ntile, 1], EPS)
                nc.vector.tensor_scalar(
                    na.bitcast(I32), nv.bitcast(I32), 1, None,
                    ALU.logical_shift_right)
                nc.vector.tensor_scalar(
                    r0[:, 0:ntile].bitcast(I32), na.bitcast(I32), -1,
                    0x5F3759DF, ALU.mult, ALU.add)
                for rin, rout in ((r0, r1), (r1, rstd)):
                    nc.vector.tensor_mul(na, rin[:, 0:ntile], rin[:, 0:ntile])
                    nc.vector.tensor_mul(nu, na, nv)
                    nc.vector.tensor_scalar(
                        nu, nu, -0.5, 1.5, ALU.mult, ALU.add)
                    nc.vector.tensor_mul(rout[:, 0:ntile], rin[:, 0:ntile], nu)
                ynrm = ynrm_pool.tile([128, TPC, 256], BF16, tag="ynrm")
                for i in range(ntile):
                    t = t0 + i
                    nc.vector.tensor_scalar(
                        ynrm[:, i, :], xch[:, t, :], mv[:, t, 0:1],
                        rstd[:, i:i + 1], ALU.subtract, ALU.mult)
                # out[r, q, p] = in[p, q*128 + r]
                base = (ch * TPC + t0) * 256
                nc.sync.dma_start(
                    out=yt[:, base:base + ntile * 256]
                    .rearrange("p (q r) -> p q r", r=128),
                    in_=ynrm[:, 0:ntile, :].rearrange("p t c -> p (t c)"),
                    transpose=True)

            # chunk 0 eagerly, in two 8-tile segments so stripes start sooner
            xcur = chunk_load(0)
            xnext = chunk_load(1)
            st_c, mv_c = stats_alloc()
            for half in range(2):
                stats_piece(xcur, st_c, mv_c, half * 8, 8)
                chunk_finish(0, xcur, mv_c, half * 8, 8)

            PIECES = [3, 3, 3, 3, 2, 2]  # next chunk's stats after stripes 2..7
            for ch in range(NCHUNK):
                nxt = ch + 1
                if nxt < NCHUNK:
                    st_n, mv_n = stats_alloc()
                for k in range(NCHUNK):
                    run_stripe(0, ch * NCHUNK + k, k % 2 == 0)
                    if nxt < NCHUNK:
                        if k == 0 and nxt + 1 < NCHUNK:
                            xfut = chunk_load(nxt + 1)
                        if 2 <= k:
                            t0 = sum(PIECES[:k - 2])
                            stats_piece(xnext, st_n, mv_n, t0, PIECES[k - 2])
                if nxt < NCHUNK:
                    chunk_finish(nxt, xnext, mv_n, 0, TPC)
                    xnext = xfut if nxt + 1 < NCHUNK else None
            # -- V stripes (no LN load: drains on DVE) --
            for g in range(NS):
                run_stripe(1, g, False)
            stripe_back(*pend, od_pool, drec_pool)

        # ---------------- phase C: projection + residual ----------------
        GT = 4                     # token tiles per group
        NG = NT // GT              # 32 groups
        hVv = hVt[:].rearrange("p (w h) -> p h w", h=HH)
        with (
            tc.tile_pool(name="pps", bufs=4, space="PSUM") as p_pool,
            tc.tile_pool(name="po", bufs=4) as po_pool,
            tc.tile_pool(name="xres", bufs=4) as xres_pool,
        ):
            xr = None
            for gi in range(NG):
                if gi % 2 == 0:
                    xr = xres_pool.tile([128, 2 * GT, 256], F32, tag="xres")
                    nc.gpsimd.dma_start(
                        out=xr[:],
                        in_=x_h[gi * GT * 128:(gi + 2) * GT * 128, :]
                        .rearrange("(t p) c -> p t c", t=2 * GT))
                xres = xr[:, (gi % 2) * GT:(gi % 2) * GT + GT, :]
                p_ps = p_pool.tile([128, GT * 256], F32, tag="pps")
                for t in range(GT):
                    i = gi * GT + t
                    nc.tensor.matmul(
                        p_ps[:, t * 256:(t + 1) * 256],
                        lhsT=hHt[:, i * 128:(i + 1) * 128],
                        rhs=wproj[:, 0:256], start=True, stop=False)
                    nc.tensor.matmul(
                        p_ps[:, t * 256:(t + 1) * 256],
                        lhsT=hVv[:, i, :],
                        rhs=wproj[:, 256:512], start=False, stop=not has_pbias)
                    if has_pbias:
                        nc.tensor.matmul(
                            p_ps[:, t * 256:(t + 1) * 256],
                            lhsT=ones[:, 0:128], rhs=bprow[:],
                            start=False, stop=True)
                po = po_pool.tile([128, GT * 256], F32, tag="po")
                nc.vector.tensor_add(
                    po[:], p_ps[:], xres.rearrange("p t c -> p (t c)"))
                og = (out_h[gi * GT * 128:(gi + 1) * GT * 128, :]
                      .rearrange("(t p) c -> p t c", t=GT))
                qdma = nc.sync if gi % 2 == 0 else nc.scalar
                qdma.dma_start(out=og,
                               in_=po[:].rearrange("p (t c) -> p t c", t=GT))

    return nc


_NC_CACHE = {}


def _get_nc(has_qbias, has_pbias):
    key = (has_qbias, has_pbias)
    if key not in _NC_CACHE:
        nc = build_nc(has_qbias, has_pbias)
        nc.finalize()
        _NC_CACHE[key] = nc
    return _NC_CACHE[key]


def kernel(x, Wqkv, bqkv, Wproj, bproj, gamma, beta, _trace=False):
    x = np.asarray(x, np.float32)
    Wqkv = np.asarray(Wqkv, np.float32)
    bqkv = np.asarray(bqkv, np.float32)
    Wproj = np.asarray(Wproj, np.float32)
    bproj = np.asarray(bproj, np.float32)
    gamma = np.asarray(gamma, np.float32)
    beta = np.asarray(beta, np.float32)

    Wg = gamma[:, None] * Wqkv                      # fold LN affine scale
    bq = beta @ Wqkv + bqkv                         # fold LN affine shift
    has_qbias = bool(np.any(bq != 0.0))
    has_pbias = bool(np.any(bproj != 0.0))

    bf = ml_dtypes.bfloat16
    wqkv_np = np.ascontiguousarray(Wg.reshape(2, 128, 768)).astype(bf)
    wproj_np = np.ascontiguousarray(Wproj.reshape(2, 128, 256)).astype(bf)
    bq_np = bq.reshape(1, 768).astype(bf)
    bp_np = bproj.reshape(1, 256).astype(bf)

    nc = _get_nc(has_qbias, has_pbias)
    in_maps = []
    for b in range(B):
        in_maps.append({
            "x": np.ascontiguousarray(x[b].reshape(T, C)),
            "wqkv": wqkv_np, "wproj": wproj_np,
            "bqkv": bq_np, "bproj": bp_np,
        })
    res = run_bass_kernel_spmd(nc, in_maps, list(range(B)), trace=_trace)
    out = np.stack([np.asarray(res.results[b]["out"]).reshape(HH, WW, C)
                    for b in range(B)])
    if _trace:
        return out.astype(np.float32), res
    return out.astype(np.float32)
